# revision 1
# baseline (speedup 1.0000x reference)
"""CharacterAwareEncoder kernel for Trainium2 (8 NeuronCores, data-parallel).

reference:
    word_embeds  = word_emb_table[word_ids]                  # [B, S, 412] gather
    char_features = sin(freqs * word_ids), 0 where id == 0   # [B, S, 100]
    out = concat([word_embeds, char_features], -1)           # [B, S, 512]

Sharding: word_ids [16, 2048] flattened to 32768 tokens, 4096 per core;
embedding table replicated. Per core: 32 tiles of 128 tokens; each tile's
rows are gathered straight into the first 412 columns of a [128, 512]
output slice via indirect DMA, the sinusoidal features are computed with
a Cody-Waite range reduction + ACT-engine Sin into the last 100 columns,
and the fused [128, 512] rows are stored contiguously.

sin accuracy: x = freq*tok <= 3168 rad.  k = int(x / 2pi) (either trunc or
round-to-nearest hardware cast works), r = ((x - k*c1) - k*c2) - k*c3 with a
3-term Cody-Waite split of 2pi, then a +-2pi range wrap (fixes any off-by-one
k) and a clamp to +-PI_SAFE so the ACT Sin table (valid on [-pi, pi]) never
sees an out-of-domain value.  Max abs error vs float64 sin ~4e-7.
"""

import numpy as np

import concourse.bacc as bacc
import concourse.bass as bass
import concourse.mybir as mybir
import concourse.tile as tile
from concourse.bass_utils import run_bass_kernel_spmd

B, S = 16, 2048
V, D, H = 32000, 412, 100
OUT_D = 512
N_CORES = 8
P = 128
T_CORE = B * S // N_CORES          # 4096 tokens per core
N_TILES = T_CORE // P              # 32 tiles of 128 tokens
CHUNK_TILES = 2                    # tiles per double-buffered SBUF chunk
N_CHUNKS = N_TILES // CHUNK_TILES  # 16
SIN_TILES = 8                      # tiles per wide sin-pipeline block
N_SIN_BLOCKS = N_TILES // SIN_TILES  # 4
W = SIN_TILES * H                  # sin-pipeline width per block (800)

_f32 = mybir.dt.float32
_i32 = mybir.dt.int32

# Cody-Waite split of 2*pi: c1/c2 keep 12 mantissa bits so k*c1, k*c2 are
# exact for k <= 505; c3 absorbs the rest (residual ~7e-15).
_TWO_PI = 2.0 * np.pi
def _split_high(v):
    f = np.float32(v)
    return (f.view(np.uint32) & np.uint32(0xFFFFF000)).view(np.float32)
C1 = float(_split_high(_TWO_PI))
C2 = float(_split_high(_TWO_PI - C1))
C3 = float(np.float32(_TWO_PI - C1 - C2))
INV2PI = float(np.float32(1.0 / _TWO_PI))
PI_F32 = float(np.float32(np.pi))
TWO_PI_F32 = float(np.float32(_TWO_PI))
PI_SAFE = float(np.nextafter(np.float32(np.pi), np.float32(0)))  # < float64 pi

_NC = {}

# "indirect": one generic SWDGE indirect DMA per 128-token tile, unpadded
#   table rows (1648 B each).
# "dma_gather": one custom InstDMAGatherAnt per chunk, table padded to 512
#   floats/row (2048 B) on host so elem_size_bytes % 256 == 0; int16 indices
#   in the wrapped [i%16, i//16] layout replicated over 8x16 partitions.
GATHER_MODE = "indirect"
SWDGE_QUEUES = 2  # split indirect gathers across SWDGE queues (parallel Q7 desc-gen)
N_PASSES = 1  # >1 only for repeat-amplification timing probes
_i16 = mybir.dt.int16


def _build_nc(mode=None):
    mode = mode or GATHER_MODE
    # Bacc (not plain Bass): its compile() pass splits multi-semaphore waits
    # into InstEventSemaphore chains — TRN2 compute instructions encode at
    # most one sync wait, and walrus refuses to legalize this itself.
    nc = bacc.Bacc("TRN2", target_bir_lowering=False,
                   num_swdge_queues=SWDGE_QUEUES)
    # consts: [0:W] freqs tiled CHUNK_TILES times, [W:W+N_TILES] token ids as f32
    consts_t = nc.dram_tensor("consts", [P, W + N_TILES], _f32, kind="ExternalInput")
    if mode == "indirect":
        ids_t = nc.dram_tensor("ids", [P, N_TILES], _i32, kind="ExternalInput")
        table_t = nc.dram_tensor("table", [V, D], _f32, kind="ExternalInput")
    elif mode == "hybrid":
        ids_t = nc.dram_tensor("ids", [P, N_TILES], _i32, kind="ExternalInput")
        idx16_t = nc.dram_tensor("idx16", [P, T_CORE // 16], _i16, kind="ExternalInput")
        table_t = nc.dram_tensor("table", [V, OUT_D], _f32, kind="ExternalInput")
    else:
        ids_t = nc.dram_tensor("idx16", [P, T_CORE // 16], _i16, kind="ExternalInput")
        table_t = nc.dram_tensor("table", [V, OUT_D], _f32, kind="ExternalInput")
    out_t = nc.dram_tensor("out", [T_CORE, OUT_D], _f32, kind="ExternalOutput")

    with tile.TileContext(nc) as tc:
        with (
            tc.tile_pool(name="const", bufs=1) as cpool,
            tc.tile_pool(name="chunks", bufs=6) as chpool,
            tc.tile_pool(name="bigch", bufs=2) as bigpool,
            tc.tile_pool(name="work", bufs=2) as wpool,
        ):
            if mode == "indirect":
                ids_sb = cpool.tile([P, N_TILES], _i32)
            elif mode == "hybrid":
                ids_sb = cpool.tile([P, N_TILES], _i32)
                idx16_sb = cpool.tile([P, T_CORE // 16], _i16)
                nc.sync.dma_start(out=idx16_sb[:], in_=idx16_t[:])
            else:
                ids_sb = cpool.tile([P, T_CORE // 16], _i16)
            nc.sync.dma_start(out=ids_sb[:], in_=ids_t[:])
            consts_sb = cpool.tile([P, W + N_TILES], _f32)
            nc.sync.dma_start(out=consts_sb[:], in_=consts_t[:])
            freqs_sb = consts_sb[:, 0:W]
            tokf = consts_sb[:, W : W + N_TILES]

            chunk_toks = CHUNK_TILES * P

            def emit_sin_block(start_tile, n_tiles):
                """6-op DVE range-reduction pipeline for n_tiles tiles.

                DVE per-op fixed costs favor wide blocks, but a wide first
                block delays the pipeline head — callers mix widths."""
                w = n_tiles * H
                tok_b = tokf[:, start_tile : start_tile + n_tiles]
                x = wpool.tile([P, W], _f32, tag="x")
                nc.vector.tensor_tensor(
                    out=x[:, 0:w].rearrange("p (j h) -> p j h", j=n_tiles),
                    in0=tok_b.to_broadcast([P, n_tiles, H]),
                    in1=freqs_sb[:, 0:w].rearrange("p (j h) -> p j h", j=n_tiles),
                    op=mybir.AluOpType.mult,
                )
                kint = wpool.tile([P, W], _i32, tag="kint")
                nc.vector.tensor_scalar(
                    out=kint[:, 0:w], in0=x[:, 0:w], scalar1=INV2PI, scalar2=None,
                    op0=mybir.AluOpType.mult,
                )
                kf = wpool.tile([P, W], _f32, tag="kf")
                nc.vector.tensor_copy(out=kf[:, 0:w], in_=kint[:, 0:w])
                r = wpool.tile([P, W], _f32, tag="r")
                nc.vector.cody_waite_cascade(
                    out=r[:, 0:w], x=x[:, 0:w], k=kf[:, 0:w], c1=C1, c2=C2, c3=C3
                )
                r2 = wpool.tile([P, W], _f32, tag="r2")
                nc.vector.add_range_wrap(
                    out=r2[:, 0:w], in_=r[:, 0:w], shift=0.0, bound=PI_F32,
                    period=TWO_PI_F32,
                )
                r3 = wpool.tile([P, W], _f32, tag="r3")
                nc.vector.tensor_scalar(
                    out=r3[:, 0:w], in0=r2[:, 0:w], scalar1=PI_SAFE, scalar2=-PI_SAFE,
                    op0=mybir.AluOpType.min, op1=mybir.AluOpType.max,
                )
                return r3

            def emit_pass():
              # narrow blocks first to prime the pipeline, wide after
              sin_plan = [CHUNK_TILES] * (SIN_TILES // CHUNK_TILES)
              while sum(sin_plan) < N_TILES:
                sin_plan.append(SIN_TILES)
              tile_block = {}  # start tile of chunk -> (r3 tile, block start)
              blocks_emitted = 0
              next_block_tile = 0

              for g in range(N_CHUNKS):
                tile0 = g * CHUNK_TILES
                if tile0 == next_block_tile:
                    n_t = sin_plan[blocks_emitted]
                    r3_b = emit_sin_block(tile0, n_t)
                    for tt in range(tile0, tile0 + n_t, CHUNK_TILES):
                        tile_block[tt] = (r3_b, tile0)
                    blocks_emitted += 1
                    next_block_tile += n_t

                ch = chpool.tile([P, CHUNK_TILES, OUT_D], _f32, tag="ch")
                if mode == "indirect":
                    # One gather per 128-token tile. NOTE: a single batched
                    # indirect DMA with a [128, k] offset AP matches CoreSim
                    # but is WRONG on hardware (the DGE reads consecutive
                    # table rows past the first offset of each partition) —
                    # keep offsets strictly [128, 1] per instruction.
                    for j in range(CHUNK_TILES):
                        t = g * CHUNK_TILES + j
                        gi = nc.gpsimd.indirect_dma_start(
                            out=ch[:, j, 0:D],
                            out_offset=None,
                            in_=table_t[:],
                            in_offset=bass.IndirectOffsetOnAxis(
                                ap=ids_sb[:, t : t + 1], axis=0
                            ),
                        )
                        if SWDGE_QUEUES > 1 and t % SWDGE_QUEUES:
                            gi.queue = f"qPoolDynamic{t % SWDGE_QUEUES}"
                else:
                    # One custom-ucode gather for the whole chunk:
                    # dst[i%128, i//128, :] = table[idx[i], :] for the
                    # chunk's 512 tokens — exactly the ch layout. The padded
                    # columns 412:512 are overwritten by the sin below.
                    nc.gpsimd.dma_gather(
                        ch[:],
                        table_t[:],
                        ids_sb[:, g * (chunk_toks // 16) : (g + 1) * (chunk_toks // 16)],
                        chunk_toks,
                        chunk_toks,
                        OUT_D,
                    )

                r3_b, bstart = tile_block[g * CHUNK_TILES]
                jj = g * CHUNK_TILES - bstart
                nc.scalar.activation(
                    out=ch[:, :, D:OUT_D],
                    in_=r3_b[:, jj * H : (jj + CHUNK_TILES) * H]
                    .rearrange("p (j h) -> p j h", j=CHUNK_TILES),
                    func=mybir.ActivationFunctionType.Sin,
                )

                # store: token g*CT*128 + j*128 + p lives at ch[p, j, :].
                # Alternate the two HWDGE rings (SP via nc.sync, ACT via
                # nc.scalar) so descriptor generation isn't serialized on
                # one engine.
                store_eng = nc.sync if g % 2 == 0 else nc.scalar
                store_eng.dma_start(
                    out=out_t[g * CHUNK_TILES * P : (g + 1) * CHUNK_TILES * P, :]
                    .rearrange("(j p) c -> p j c", p=P),
                    in_=ch[:],
                )

            def emit_pass_hybrid():
              # Tiles 0..15: fine-grained indirect chunks (full padded rows).
              # Tiles 16..31: two 1024-row dma_gather super-chunks — cuts the
              # Pool engine's serial gather dispatches from 32 to 18.
              sin_plan = [CHUNK_TILES] * (SIN_TILES // CHUNK_TILES)
              while sum(sin_plan) < N_TILES // 2:
                sin_plan.append(SIN_TILES)
              tile_block = {}
              blocks_emitted = 0
              next_block_tile = 0
              for g in range((N_TILES // 2) // CHUNK_TILES):
                tile0 = g * CHUNK_TILES
                if tile0 == next_block_tile:
                    n_t = sin_plan[blocks_emitted]
                    r3_b = emit_sin_block(tile0, n_t)
                    for tt in range(tile0, tile0 + n_t, CHUNK_TILES):
                        tile_block[tt] = (r3_b, tile0)
                    blocks_emitted += 1
                    next_block_tile += n_t
                ch = chpool.tile([P, CHUNK_TILES, OUT_D], _f32, tag="ch")
                for j in range(CHUNK_TILES):
                    t = tile0 + j
                    gi = nc.gpsimd.indirect_dma_start(
                        out=ch[:, j, :],
                        out_offset=None,
                        in_=table_t[:],
                        in_offset=bass.IndirectOffsetOnAxis(
                            ap=ids_sb[:, t : t + 1], axis=0
                        ),
                    )
                    if SWDGE_QUEUES > 1 and t % SWDGE_QUEUES:
                        gi.queue = f"qPoolDynamic{t % SWDGE_QUEUES}"
                r3_b, bstart = tile_block[tile0]
                jj = tile0 - bstart
                nc.scalar.activation(
                    out=ch[:, :, D:OUT_D],
                    in_=r3_b[:, jj * H : (jj + CHUNK_TILES) * H]
                    .rearrange("p (j h) -> p j h", j=CHUNK_TILES),
                    func=mybir.ActivationFunctionType.Sin,
                )
                store_eng = nc.sync if g % 2 == 0 else nc.scalar
                store_eng.dma_start(
                    out=out_t[tile0 * P : (tile0 + CHUNK_TILES) * P, :]
                    .rearrange("(j p) c -> p j c", p=P),
                    in_=ch[:],
                )
              sc_toks = SIN_TILES * P
              for s in range((N_TILES // 2) // SIN_TILES):
                tile0 = N_TILES // 2 + s * SIN_TILES
                big = bigpool.tile([P, SIN_TILES, OUT_D], _f32, tag="big")
                nc.gpsimd.dma_gather(
                    big[:],
                    table_t[:],
                    idx16_sb[:, tile0 * P // 16 : (tile0 * P + sc_toks) // 16],
                    sc_toks,
                    sc_toks,
                    OUT_D,
                )
                r3_b = emit_sin_block(tile0, SIN_TILES)
                nc.scalar.activation(
                    out=big[:, :, D:OUT_D],
                    in_=r3_b[:, 0 : SIN_TILES * H]
                    .rearrange("p (j h) -> p j h", j=SIN_TILES),
                    func=mybir.ActivationFunctionType.Sin,
                )
                # split the 2MB store into 512KB sub-stores alternating both
                # HWDGE rings — one big store serializes ~6.3us on one ring
                # at the kernel tail
                for q in range(0, SIN_TILES, CHUNK_TILES):
                    r0 = (tile0 + q) * P
                    eng = nc.sync if (s + q // CHUNK_TILES) % 2 == 0 else nc.scalar
                    eng.dma_start(
                        out=out_t[r0 : r0 + CHUNK_TILES * P, :]
                        .rearrange("(j p) c -> p j c", p=P),
                        in_=big[:, q : q + CHUNK_TILES, :],
                    )

            for _ in range(N_PASSES):
                if mode == "hybrid":
                    emit_pass_hybrid()
                else:
                    emit_pass()
    nc.compile()
    return nc


def _get_nc(mode=None):
    mode = mode or GATHER_MODE
    if mode not in _NC:
        _NC[mode] = _build_nc(mode)
    return _NC[mode]


def make_in_maps(word_ids, word_emb_table, mode=None):
    mode = mode or GATHER_MODE
    ids = np.ascontiguousarray(np.asarray(word_ids)).astype(np.int32).reshape(-1)
    table = np.ascontiguousarray(np.asarray(word_emb_table, dtype=np.float32))
    if mode != "indirect":
        padded = np.zeros((V, OUT_D), np.float32)
        padded[:, 0:D] = table
        table = padded
    freqs_row = np.tile(np.arange(H, dtype=np.float32) / np.float32(1000.0),
                        W // H)  # [W]

    in_maps = []
    for c in range(N_CORES):
        shard = ids[c * T_CORE : (c + 1) * T_CORE]
        ids_in = np.ascontiguousarray(shard.reshape(N_TILES, P).T)  # [P, N_TILES]
        consts = np.empty((P, W + N_TILES), np.float32)
        consts[:, 0:W] = freqs_row
        consts[:, W:] = ids_in.astype(np.float32)  # exact, ids < 2^24
        m = {"consts": consts, "table": table}
        if mode in ("indirect", "hybrid"):
            m["ids"] = ids_in
        if mode != "indirect":
            # wrapped int16 layout: shard token i at [i % 16, i // 16],
            # replicated over the 8 groups of 16 partitions (one per Q7 core)
            base = shard.astype(np.int16).reshape(T_CORE // 16, 16).T  # [16, n/16]
            m["idx16"] = np.ascontiguousarray(np.tile(base, (8, 1)))
        in_maps.append(m)
    return in_maps


def kernel(word_ids, word_emb_table):
    nc = _get_nc()
    in_maps = make_in_maps(word_ids, word_emb_table)
    res = run_bass_kernel_spmd(nc, in_maps, core_ids=list(range(N_CORES)))
    out = np.concatenate([r["out"] for r in res.results], axis=0)
    return out.reshape(B, S, OUT_D)

